# revision 76
# baseline (speedup 1.0000x reference)
"""Multi-head attention (B=2, L=2048, dim=1024, 16 heads) on 8 Trainium2 cores.

Sharding: 8 cores = 2 (batch) x 4 (head groups of 4 heads). Each core runs an
identical Bass program on its own slice (SPMD, no collectives); the host sums
the 4 per-head-group partial projection outputs per batch and adds the bias.

Per-core dataflow (bf16 matmul operands, fp32 PSUM accumulation):
  xT [1024, 2048]  (x[b] transposed, channel-major, bf16)
  V token-major [128 tok, 4 heads, 64+1] (ones column fused for the softmax
    denominator), qT/kT feature-major [128 (2 heads x 64d), 2048]
  ST[k, q] = kT.T @ qT    (K=64 contraction per head)
  PT = exp(ST / 8)        (ScalarE, PSUM -> SBUF bf16; no max-subtraction
                           needed: |S/8| <= ~7 so exp is safely in range)
  OT[d, q] += V.T @ PT    (M=65: row 64 accumulates the softmax denominator)
  1/denom = exp(-ln(denom))       (Scalar engine, one act table with exp)
  rbc = ones.T @ recip_row        (PE broadcast into PSUM, no DMA latency)
  OT_norm = OT * rbc              (DVE; DMA shifts odd head to rows 64-127)
  out[tok, c] = OT_norm.T @ wpT   (contract 4 heads x 64 channels)

Scheduling: everything is one fused phase. The scores+exp stream runs two
key-blocks ahead of PV consumption globally (crossing head-pair boundaries)
so the Scalar engine's exp latency is always hidden; QKV-projection and
out-projection work units are interleaved between key blocks via a static
filler schedule with each consumer placed after its producer chain's real
latency; each pair's normalize is split in deferred stages drained early in
the NEXT pair's sweep. The last query chunk runs its head pairs in reverse
order so the final epilogue chain overlaps the other pair's attention.
Measured on 8 axon TRN2 cores: ~237us HW exec (baseline of this session:
~265us), scale-relative absmax error ~6.6e-3 vs the fp32 reference.
"""

import os
import numpy as np

B, L, C = 2, 2048, 1024
H, D = 16, 64
HL = 4            # heads per core (local)
PAIRS = 2         # head pairs per core
CT = C // 128     # 8 contraction tiles for the projections
TOK = L // 128    # 16 key-token tiles
QW = 512          # query tile width
QS = L // QW      # 4 query tiles
NCORES = 8

_cache = {}


def _build_nc():
    import concourse.bass as bass
    import concourse.mybir as mybir
    import concourse.tile as tile
    from concourse import bacc

    F32 = mybir.dt.float32
    BF16 = mybir.dt.bfloat16
    EXP = mybir.ActivationFunctionType.Exp
    COPY = mybir.ActivationFunctionType.Copy
    LN = mybir.ActivationFunctionType.Ln

    nc = bacc.Bacc("TRN2", target_bir_lowering=False, debug=False,
                   num_devices=NCORES)

    # This kernel's activations are {exp, ln, copy} and all three live in
    # the "natural_log_exp_and_others" table, but the default assignment
    # binds plain exp to "exp_and_others", so each epilogue's ln/exp pair
    # ping-pongs tables (2x 1283ns ACT_TABLE_LOAD per head pair). Keep the
    # table list order (positions are the global act_func_set_id walrus
    # indexes into act_info.json) but blank out every other table's
    # function set so the assignment resolves everything to the one table.
    import types
    import bass_rust as _bass_rust
    from concourse.hw_specs import get_activation_tables

    def _act_loads_one_table(self):
        has_act = any(
            isinstance(i, mybir.InstActivation)
            for b in self.main_func.blocks
            for i in b.instructions
        )
        if not has_act:
            return
        tables = [
            (name, funcs if name == "natural_log_exp_and_others" else set())
            for name, funcs in get_activation_tables(self.m.arch).items()
        ]
        _bass_rust.insert_act_table_loads(self, tables)

    nc.insert_act_table_loads = types.MethodType(_act_loads_one_table, nc)

    xT = nc.declare_dram_parameter("xT", [C, L], BF16, isOutput=False)
    wT = nc.declare_dram_parameter("wT", [C, 3 * HL * D], BF16, isOutput=False)
    wpT = nc.declare_dram_parameter("wpT", [HL * D, C], BF16, isOutput=False)
    out = nc.declare_dram_parameter("out", [L, C], F32, isOutput=True)

    with tile.TileContext(nc) as tc:
        from contextlib import ExitStack
        with ExitStack() as ctx:
            xpool = ctx.enter_context(tc.tile_pool(name="x", bufs=1))
            wpool = ctx.enter_context(tc.tile_pool(name="w", bufs=1))
            wppool = ctx.enter_context(tc.tile_pool(name="wp", bufs=1))
            qkpool = ctx.enter_context(tc.tile_pool(name="qk", bufs=1))
            vpool = ctx.enter_context(tc.tile_pool(name="v", bufs=1))
            ptpool = ctx.enter_context(tc.tile_pool(name="pt", bufs=8))
            otpool = ctx.enter_context(tc.tile_pool(name="ot", bufs=1))
            obpool = ctx.enter_context(tc.tile_pool(name="ob", bufs=4))
            ocpool = ctx.enter_context(tc.tile_pool(name="oc", bufs=2))
            rpool = ctx.enter_context(tc.tile_pool(name="r", bufs=2))
            psS = ctx.enter_context(tc.tile_pool(name="psS", bufs=2, space="PSUM"))
            psOT = ctx.enter_context(tc.tile_pool(name="psOT", bufs=2, space="PSUM"))
            psF = ctx.enter_context(tc.tile_pool(name="psF", bufs=2, space="PSUM"))

            # ---- input loads ------------------------------------------------
            # one DMA per tile: descriptors are generated per partition row
            # and round-robin across all 16 queues, so fewer/larger DMAs
            # minimize per-descriptor overhead and total load latency
            x_t, w_t = [], []
            for i in range(CT):
                tx = xpool.tile([128, L], BF16, name=f"x{i}", tag=f"x{i}")
                nc.sync.dma_start(out=tx, in_=xT[128 * i:128 * (i + 1), :])
                x_t.append(tx)
            for i in range(CT):
                tw = wpool.tile([128, 3 * HL * D], BF16, name=f"w{i}", tag=f"w{i}")
                nc.sync.dma_start(out=tw, in_=wT[128 * i:128 * (i + 1), :])
                w_t.append(tw)
            wp_t = []
            pending_fin = []
            ones_s = vpool.tile([128, HL, 1], F32, name="ones_s", tag="ones_s")
            nc.vector.memset(ones_s, 1.0)
            # row 64 of this tile is the stationary ones-column used to
            # broadcast the reciprocal row across 64 PSUM partitions
            ones_r = vpool.tile([65, 64], BF16, name="ones_r", tag="ones_r")
            nc.vector.memset(ones_r, 1.0)

            # persistent SBUF tensors
            v_t = [None] * TOK
            qk_t = {}
            for p in range(PAIRS):
                for nm in ("q", "k"):
                    qk_t[(nm, p)] = qkpool.tile(
                        [128, L], BF16, name=f"{nm}{p}", tag=f"{nm}{p}")
            # per-(pair, qs) tiles: the tile dependency tracker is
            # coarse-grained, so a single [128, L] tile would serialize the
            # out-projection of chunk qs-1 behind the epilogue of chunk qs
            ot_sb = [[otpool.tile([128, QW], BF16, name=f"otp{p}q{qs}",
                                  tag=f"otp{p}q{qs}")
                      for qs in range(QS)] for p in range(PAIRS)]

            # ---- work units -------------------------------------------------
            def unit_v(t):
                ps = psF.tile([128, HL * D], F32, name="psv", tag="ps")
                for c in range(CT):
                    nc.tensor.matmul(
                        ps,
                        lhsT=x_t[c][:, 128 * t:128 * (t + 1)],
                        rhs=w_t[c][:, 2 * HL * D:3 * HL * D],
                        start=(c == 0), stop=(c == CT - 1),
                    )
                vt = vpool.tile([128, HL, D + 1], BF16, name=f"v{t}", tag=f"v{t}")
                nc.vector.tensor_copy(out=vt[:, :, D:D + 1], in_=ones_s)
                nc.vector.tensor_copy(
                    out=vt[:, :, 0:D],
                    in_=ps.rearrange("p (h d) -> p h d", h=HL),
                )
                v_t[t] = vt

            def unit_qk(nm, p, ns):
                j = 0 if nm == "q" else 1
                ps = psF.tile([128, QW], F32, name="psqk", tag="ps")
                for c in range(CT):
                    nc.tensor.matmul(
                        ps,
                        lhsT=w_t[c][:, j * HL * D + 128 * p:
                                    j * HL * D + 128 * (p + 1)],
                        rhs=x_t[c][:, QW * ns:QW * (ns + 1)],
                        start=(c == 0), stop=(c == CT - 1),
                    )
                nc.vector.tensor_copy(
                    out=qk_t[(nm, p)][:, QW * ns:QW * (ns + 1)], in_=ps)

            def unit_proj(qs, tt, nh, last=False):
                t = 4 * qs + tt
                ps = psF.tile([128, QW], F32, name="psp", tag="ps")
                # contract the staler pair first so the unit's first matmul
                # never waits a just-finished normalize chain (pairs run
                # (0,1) within qs 0-2, but qs3 runs (1,0))
                order = (1, 0) if last else (0, 1)
                for p2 in order:
                    nc.tensor.matmul(
                        ps,
                        lhsT=ot_sb[p2][qs][:, 128 * tt:128 * (tt + 1)],
                        rhs=wp_t[p2][:, QW * nh:QW * (nh + 1)],
                        start=(p2 == order[0]), stop=(p2 == order[1]),
                    )
                ob = obpool.tile([128, QW], F32, name="ob", tag="ob")
                if last and nh == 0:
                    # tail: split the staging copies across ACT and DVE so
                    # the eight of them drain in half the serial time
                    nc.scalar.activation(out=ob, in_=ps, func=COPY)
                else:
                    nc.vector.tensor_copy(out=ob, in_=ps)
                # two half-DMAs -> two queues: halves the store latency
                nc.sync.dma_start(
                    out=out[128 * t:128 * (t + 1),
                            QW * nh:QW * nh + QW // 2],
                    in_=ob[:, 0:QW // 2])
                nc.sync.dma_start(
                    out=out[128 * t:128 * (t + 1),
                            QW * nh + QW // 2:QW * (nh + 1)],
                    in_=ob[:, QW // 2:QW])

            # ---- static filler schedule ------------------------------------
            # fillers[(qs, p, kb)] -> list of closures emitted before that
            # attention iteration's score matmuls.
            fillers = {}

            def addf(qs, p, kb, fn):
                fillers.setdefault((qs, p, kb), []).append(fn)

            # (0,0): V7..V15 just-in-time (Vt needed by PV at kb=t), K/Q next
            v_slots = {0: 6, 1: 7, 3: 8, 4: 9, 5: 10, 7: 11, 8: 12, 9: 13,
                       11: 14, 12: 15}
            for kb, t in v_slots.items():
                addf(0, 0, kb, (lambda t=t: unit_v(t)))
            addf(0, 0, 2, lambda: unit_qk("k", 0, 1))
            addf(0, 0, 6, lambda: unit_qk("k", 0, 2))
            addf(0, 0, 10, lambda: unit_qk("k", 0, 3))
            addf(0, 0, 13, lambda: unit_qk("k", 1, 1))
            addf(0, 0, 14, lambda: unit_qk("q", 0, 1))
            addf(0, 1, 0, lambda: unit_qk("k", 1, 2))
            addf(0, 1, 2, lambda: unit_qk("k", 1, 3))
            addf(0, 1, 4, lambda: unit_qk("q", 1, 1))
            # proj(qs-1) runs during the SECOND pair of sweep qs — a full
            # pair-sweep (~17us) after epilogue(qs-1, p1) was issued, so its
            # normalize chain (ACT recip + DMA broadcast + muls + shift DMA,
            # ~15us end-to-end through loaded queues) has fully drained and
            # the in-order PE never stalls on a filler.
            # proj(qs-1) runs in the FIRST pair-window of sweep qs (kb>=9,
            # after the previous epilogue's ~9us normalize chain drains), so
            # its 2MB of output stores never saturate the DMA queues while
            # the LAST pair's epilogue needs them (that was a ~17us tail).
            for qs in (1, 2):
                for u in range(8):
                    tt, nh = divmod(u, 2)
                    fn = (lambda qs=qs, tt=tt, nh=nh: unit_proj(qs - 1, tt, nh))
                    if u < 6:
                        addf(qs, 0, 9 + u, fn)
                    else:
                        # two units land right after the next pair's ACT
                        # recip interruptions (kb2/kb6 drains) to keep the
                        # PE fed while the exp stream catches up
                        addf(qs, 1, 3 + 2 * (u - 6), fn)
            addf(1, 0, 7, lambda: unit_qk("q", 0, 2))
            addf(1, 1, 2, lambda: unit_qk("q", 1, 2))
            addf(2, 0, 7, lambda: unit_qk("q", 1, 3))
            addf(2, 1, 2, lambda: unit_qk("q", 0, 3))
            # qs=3 runs pair 1 first, then pair 0; proj(2) spread the same way
            for u in range(8):
                tt, nh = divmod(u, 2)
                fn = (lambda tt=tt, nh=nh: unit_proj(2, tt, nh))
                if u < 6:
                    addf(3, 1, 9 + u, fn)
                else:
                    addf(3, 0, 3 + 2 * (u - 6), fn)

            # ---- attention + epilogue --------------------------------------
            def epilogue(qs, p, ot_a, ot_b, c0=0, cw=QW, tail=False):
                # Immediate part: evacuate the PSUM accumulators (frees the
                # psOT slots for the next pair). Everything else is deferred
                # into the next pair's sweep so the pair boundary never
                # stalls PE or interrupts the ACT exp stream.
                oc = ocpool.tile([65, 2 * cw], F32, name="oc", tag="oc")
                nc.vector.tensor_copy(out=oc[:, 0:cw], in_=ot_a)
                nc.vector.tensor_copy(out=oc[:, cw:2 * cw], in_=ot_b)

                lns = rpool.tile([65, 2 * cw], F32, name="lns", tag="lns")
                rsb = rpool.tile([65, 2 * cw], BF16, name="rsb", tag="rsb")

                def stage1a():
                    # ln(denom) on the Scalar engine (same act table as the
                    # exp stream -> no table reloads). For the final pair the
                    # lns read the PSUM rows directly (no oc-copy wait) since
                    # the ACT queue is empty at the tail.
                    if tail:
                        nc.scalar.activation(
                            out=lns[64:65, 0:cw], in_=ot_a[64:65, :], func=LN)
                        nc.scalar.activation(
                            out=lns[64:65, cw:2 * cw], in_=ot_b[64:65, :],
                            func=LN)
                    else:
                        nc.scalar.activation(
                            out=lns[64:65, :], in_=oc[64:65, :], func=LN)

                def stage1b():
                    # separate drain point: each ACT interruption stays
                    # smaller than the score-buffer slack, so the exp
                    # stream never falls behind enough to stall the pump
                    nc.scalar.activation(
                        out=rsb[64:65, :], in_=lns[64:65, :], func=EXP,
                        scale=-1.0)

                def stage2():
                    # broadcast 1/denom across 64 partitions via a tiny
                    # ones-column matmul into PSUM (no DMA queue latency),
                    # then normalize on DVE; a DMA shifts the odd head into
                    # partitions 64-127 of the bf16 staging tile
                    rbc_a = psF.tile([64, cw], F32, name="rbca", tag="ps")
                    nc.tensor.matmul(
                        rbc_a, lhsT=ones_r[64:65, 0:64],
                        rhs=rsb[64:65, 0:cw], start=True, stop=True)
                    rbc_b = psF.tile([64, cw], F32, name="rbcb", tag="ps")
                    nc.tensor.matmul(
                        rbc_b, lhsT=ones_r[64:65, 0:64],
                        rhs=rsb[64:65, cw:2 * cw], start=True, stop=True)
                    nc.vector.tensor_mul(
                        out=ot_sb[p][qs][0:64, c0:c0 + cw],
                        in0=oc[0:64, 0:cw], in1=rbc_a)
                    stg = rpool.tile([64, cw], BF16, name="stg", tag="stg")
                    nc.vector.tensor_mul(
                        out=stg, in0=oc[0:64, cw:2 * cw], in1=rbc_b)
                    nch = max(1, cw // 128)
                    for k in range(nch):
                        rl = 64 // nch
                        nc.sync.dma_start(
                            out=ot_sb[p][qs][64 + rl * k:64 + rl * (k + 1),
                                             c0:c0 + cw],
                            in_=stg[rl * k:rl * (k + 1), :])
                if tail:
                    stage1a()
                    stage1b()
                    stage2()
                else:
                    pending_fin.append((stage1a, stage1b, stage2))

            def emit_scores(qs, p, kb):
                kT = qk_t[("k", p)]
                qT = qk_t[("q", p)]
                st = psS.tile([128, 2, QW], F32, name="st", tag="st")
                nc.tensor.matmul(
                    st[:, 0, :],
                    lhsT=kT[0:64, 128 * kb:128 * (kb + 1)],
                    rhs=qT[0:64, QW * qs:QW * (qs + 1)],
                    start=True, stop=True,
                )
                nc.tensor.matmul(
                    st[:, 1, :],
                    lhsT=kT[64:128, 128 * kb:128 * (kb + 1)],
                    rhs=qT[64:128, QW * qs:QW * (qs + 1)],
                    start=True, stop=True,
                )
                pt = ptpool.tile([128, 2, QW], BF16, name="pt", tag="pt")
                nc.scalar.activation(out=pt, in_=st, func=EXP, scale=0.125)
                return pt

            # the scores+exp stream runs two key-blocks ahead of the PV
            # consumption GLOBALLY (crossing pair boundaries), so the exp
            # latency is always hidden and the PE never micro-stalls (each
            # stall also resets the Tensor engine's clock ramp)
            pair_seq = [(0, 0), (0, 1), (1, 0), (1, 1),
                        (2, 0), (2, 1), (3, 1), (3, 0)]
            score_seq = [(pi, kb) for pi in range(len(pair_seq))
                         for kb in range(TOK)]
            pts = {}
            score_pos = [0]

            def pump():
                if score_pos[0] < len(score_seq):
                    pi, kb = score_seq[score_pos[0]]
                    score_pos[0] += 1
                    qs, p = pair_seq[pi]
                    pts[(pi, kb)] = emit_scores(qs, p, kb)

            def attention(pi):
                qs, p = pair_seq[pi]
                ot_a = psOT.tile([65, QW], F32, name="ot_a", tag="ot")
                ot_b = psOT.tile([65, QW], F32, name="ot_b", tag="ot")
                for kb in range(TOK):
                    if kb == 2:
                        for e in pending_fin:
                            e[0]()
                    elif kb == 4:
                        for e in pending_fin:
                            e[1]()
                    elif kb == 6:
                        while pending_fin:
                            pending_fin.pop(0)[2]()
                    for fn in fillers.pop((qs, p, kb), ()):
                        fn()
                    pump()
                    pt = pts.pop((pi, kb))
                    nc.tensor.matmul(
                        ot_a,
                        lhsT=v_t[kb][:, 2 * p, :],
                        rhs=pt[:, 0, :],
                        start=(kb == 0), stop=(kb == TOK - 1),
                    )
                    nc.tensor.matmul(
                        ot_b,
                        lhsT=v_t[kb][:, 2 * p + 1, :],
                        rhs=pt[:, 1, :],
                        start=(kb == 0), stop=(kb == TOK - 1),
                    )
                epilogue(qs, p, ot_a, ot_b, tail=(pi == len(pair_seq) - 1))

            # ---- prologue: enough V/K/Q for the first pair-sweep -----------
            # K/Q of pair 0 first so the Scalar engine's exp stream (the
            # near-critical engine) starts as early as possible
            unit_qk("k", 0, 0)
            unit_qk("q", 0, 0)
            pump()
            pump()
            for t in range(6):
                unit_v(t)
            unit_qk("k", 1, 0)
            unit_qk("q", 1, 0)
            # wp loads deferred here: they are only needed by qs1's proj
            # fillers, so they stay out of the startup DMA set that gates
            # the first matmuls (the DMA-queue waits are conservative)
            for p in range(PAIRS):
                t = wppool.tile([128, C], BF16, name=f"wp{p}", tag=f"wp{p}")
                nc.sync.dma_start(out=t, in_=wpT[2 * D * p:2 * D * (p + 1), :])
                wp_t.append(t)

            # ---- main loop --------------------------------------------------
            for pi in range(len(pair_seq)):
                attention(pi)

            # tail: finish the last pair's normalize, then the out-projection
            # of the last query chunk
            for e in pending_fin:
                e[0]()
            for e in pending_fin:
                e[1]()
            while pending_fin:
                pending_fin.pop(0)[2]()
            for u in range(8):
                tt, nh = divmod(u, 2)
                unit_proj(QS - 1, tt, nh, last=True)

    nc.compile()
    return nc


def _get_nc():
    if "nc" not in _cache:
        _cache["nc"] = _build_nc()
    return _cache["nc"]


def kernel(x, w_qkv, w_proj, b_proj):
    import ml_dtypes
    from concourse.bass_utils import run_bass_kernel_spmd

    x = np.asarray(x, dtype=np.float32)
    w_qkv = np.asarray(w_qkv, dtype=np.float32)
    w_proj = np.asarray(w_proj, dtype=np.float32)
    b_proj = np.asarray(b_proj, dtype=np.float32)

    nc = _get_nc()
    in_maps = []
    for core in range(NCORES):
        b, g = divmod(core, 4)
        rows = np.concatenate([
            np.arange(C * j + HL * D * g, C * j + HL * D * (g + 1))
            for j in range(3)
        ])
        in_maps.append({
            "xT": np.ascontiguousarray(x[b].T).astype(ml_dtypes.bfloat16),
            "wT": np.ascontiguousarray(w_qkv[rows].T).astype(ml_dtypes.bfloat16),
            "wpT": np.ascontiguousarray(
                w_proj[:, HL * D * g:HL * D * (g + 1)].T).astype(ml_dtypes.bfloat16),
        })

    res = run_bass_kernel_spmd(
        nc, in_maps, list(range(NCORES)),
        trace=bool(os.environ.get("KERNEL_TRACE")),
    )
    _cache["last_results"] = res

    out = np.empty((B, L, C), dtype=np.float32)
    for b in range(B):
        acc = res.results[4 * b]["out"].astype(np.float32)
        for g in range(1, 4):
            acc = acc + res.results[4 * b + g]["out"]
        out[b] = acc + b_proj[None, :]
    return out
